# revision 1
# baseline (speedup 1.0000x reference)
"""Trainium2 Bass kernel for full-dim attention — bf16 resident v3.

Folded algorithm (scores = x M x.T / 32 with M = wq.T wk;
out = (p x) W2.T with W2 = wo wv). All matmul operands bf16 (same PE
rate as fp32r, half the SBUF/DMA), which lets every input stay fully
resident: x.T [128,8,2048], x natural [128,16,1024], mT, w2T, uT, pT
(both halves), px. Total ~21 MB SBUF, 12 MB DMA in, all front-loaded —
no mid-kernel streaming bursts (the v2 restructure showed the fp32r
xTk/xn re-streams run at ~270 GB/s demand and stall the PE).

Stage order (PE): uT(q0) S0 uT(q1) Z0 px0 S1 out0 Z1 px1 out1 — every
cross-engine handoff (psum evac, exp, S1 sums, Z) hides behind a PE
stage that doesn't depend on it.
"""

import sys

if "/opt/trn_rl_repo" not in sys.path:
    sys.path.insert(0, "/opt/trn_rl_repo")

import numpy as np
import ml_dtypes

N_CORES = 8
P = 128

_BUILD_CACHE = {}


def _build(S, D, SQ):
    import concourse.mybir as mybir
    import concourse.tile as tile
    from concourse import bacc

    key = (S, D, SQ)
    if key in _BUILD_CACHE:
        return _BUILD_CACHE[key]

    dt = mybir.dt
    DS = D // P           # d subtiles (8)
    SKT = S // P          # key tiles (16)
    SQT = SQ // P         # query tiles (8)
    NB = 512
    NBg = min(NB, D)      # 512
    NH = max(1, SQ // NB)          # query halves (2)
    HW_ = SQ // NH                 # queries per half (512)
    HT = HW_ // P                  # query tiles per half (4)
    INV_SQRT_D = 1.0 / float(np.sqrt(np.float32(D)))

    nc = bacc.Bacc(None, target_bir_lowering=False, debug=False)

    bf = dt.bfloat16
    xT_d = nc.dram_tensor("xT", [P, DS, S], bf, kind="ExternalInput")
    xn_d = nc.dram_tensor("xn", [P, SKT, D], bf, kind="ExternalInput")
    mT_d = nc.dram_tensor("mT", [P, DS, D], bf, kind="ExternalInput")
    w2T_d = nc.dram_tensor("w2T", [P, DS, D], bf, kind="ExternalInput")
    y_d = nc.dram_tensor("y", [SQ, D], dt.float32, kind="ExternalOutput")

    with tile.TileContext(nc) as tc:
        with (
            tc.tile_pool(name="xT", bufs=1) as xT_pool,
            tc.tile_pool(name="xn", bufs=1) as xn_pool,
            tc.tile_pool(name="mT", bufs=1) as mT_pool,
            tc.tile_pool(name="w2T", bufs=1) as w2T_pool,
            tc.tile_pool(name="uT", bufs=1) as uT_pool,
            tc.tile_pool(name="pT", bufs=1) as pT_pool,
            tc.tile_pool(name="px", bufs=1) as px_pool,
            tc.tile_pool(name="stat", bufs=1) as stat_pool,
            tc.tile_pool(name="outsb", bufs=2) as out_pool,
            tc.tile_pool(name="ps", bufs=6, space="PSUM") as ps_pool,
            tc.tile_pool(name="zps", bufs=2, space="PSUM") as z_pool,
        ):
            xT = xT_pool.tile([P, DS, S], bf)
            xn = xn_pool.tile([P, SKT, D], bf)
            mT = mT_pool.tile([P, DS, D], bf)
            w2T = w2T_pool.tile([P, DS, D], bf)

            S1 = stat_pool.tile([P, SQ], dt.float32)
            # ones (col 0) and 1/Z per query tile (cols 8..) share one tile
            zs = stat_pool.tile([P, 8 + SQT], dt.float32, name="zs")

            # PE warmup; memset on GpSimd which wakes earliest
            wrm = stat_pool.tile([P, P], bf, name="wrm")
            nc.gpsimd.memset(wrm[:], 0.0)
            nc.vector.memset(zs[:, 0:1], 1.0)
            wps = z_pool.tile([P, P], dt.float32, tag="zp", name="wps")
            for i in range(32):
                nc.tensor.matmul(wps[:], wrm[:], wrm[:], start=True, stop=True)

            # DMA in consumption order: {mT half, xT query-half} pairs feed
            # the uT q0 sweep; mT rest the dt>=4 groups; xT rest feeds S0's
            # kt>=4 lhsT slices; xn feeds px0 (~70us in); w2T out0 (~105us).
            MH = min(4, DS) * P
            XH = min(NB, SQ)
            for ds in range(DS):
                nc.sync.dma_start(mT[:, ds, :MH], mT_d[:, ds, :MH])
                nc.sync.dma_start(xT[:, ds, :XH], xT_d[:, ds, :XH])
            for ds in range(DS):
                nc.sync.dma_start(mT[:, ds, MH:], mT_d[:, ds, MH:])
            for ds in range(DS):
                nc.sync.dma_start(xT[:, ds, XH:SQ], xT_d[:, ds, XH:SQ])
            for ds in range(DS):
                nc.sync.dma_start(xT[:, ds, SQ:], xT_d[:, ds, SQ:])
            for t in range(SKT):
                nc.sync.dma_start(xn[:, t, :], xn_d[:, t, :])
            for ds in range(DS):
                nc.sync.dma_start(w2T[:, ds, :], w2T_d[:, ds, :])

            uT = uT_pool.tile([P, DS, SQ], bf)
            NBq = min(NB, SQ)
            DTG = min(4, DS)

            def ut_half(qb):
                # uT[d', sq_qb] = sum_d mT[d, d'] xT[d, sq_qb]
                for g in range(DS // DTG):
                    grp = range(g * DTG, (g + 1) * DTG)
                    pss = [ps_pool.tile([P, NBq], dt.float32, tag="ps",
                                        name=f"ps_u{qb}_{g}_{j}")
                           for j in range(DTG)]
                    for ds in range(DS):
                        for j, dt_ in enumerate(grp):
                            nc.tensor.matmul(
                                pss[j][:], mT[:, ds, dt_ * P:(dt_ + 1) * P],
                                xT[:, ds, qb * NBq:(qb + 1) * NBq],
                                start=(ds == 0), stop=(ds == DS - 1),
                            )
                    for j, dt_ in enumerate(grp):
                        nc.any.tensor_copy(
                            uT[:, dt_, qb * NBq:(qb + 1) * NBq], pss[j][:])

            pT = pT_pool.tile([P, SKT, SQ], bf)

            def scores_half(h):
                hq = h * HW_
                for skt in range(SKT):
                    ps1 = ps_pool.tile([P, HW_], dt.float32, tag="ps",
                                       name=f"ps_s{h}_{skt}")
                    for ds in range(DS):
                        nc.tensor.matmul(
                            ps1[:], xT[:, ds, skt * P:(skt + 1) * P],
                            uT[:, ds, hq:hq + HW_],
                            start=(ds == 0), stop=(ds == DS - 1),
                        )
                    nc.scalar.activation(
                        pT[:, skt, hq:hq + HW_], ps1[:],
                        mybir.ActivationFunctionType.Exp, scale=INV_SQRT_D,
                    )
                    if skt == 0:
                        nc.vector.tensor_copy(S1[:, hq:hq + HW_],
                                              pT[:, 0, hq:hq + HW_])
                    else:
                        nc.vector.tensor_add(S1[:, hq:hq + HW_],
                                             S1[:, hq:hq + HW_],
                                             pT[:, skt, hq:hq + HW_])

            def z_half(h):
                for t in range(HT):
                    sqt = h * HT + t
                    zp = z_pool.tile([P, 1], dt.float32, tag="zp",
                                     name=f"zp{sqt}")
                    nc.tensor.matmul(zp[:], S1[:, sqt * P:(sqt + 1) * P],
                                     zs[:, 0:1], start=True, stop=True)
                    nc.vector.reciprocal(zs[:, 8 + sqt:9 + sqt], zp[:])

            px = px_pool.tile([P, DS, SQ], bf)

            def px_half(h):
                # pxT[d, sq_h] = sum_sk xn[sk, d] pT[sk, sq_h]
                hq = h * HW_
                for dt_ in range(DS):
                    ps2 = ps_pool.tile([P, HW_], dt.float32, tag="ps",
                                       name=f"ps_c{h}_{dt_}")
                    for skt in range(SKT):
                        nc.tensor.matmul(
                            ps2[:], xn[:, skt, dt_ * P:(dt_ + 1) * P],
                            pT[:, skt, hq:hq + HW_],
                            start=(skt == 0), stop=(skt == SKT - 1),
                        )
                    nc.any.tensor_copy(px[:, dt_, hq:hq + HW_], ps2[:])

            def emit_out_scale(sqt, gb, ps):
                ot = out_pool.tile([P, NBg], dt.float32, tag="ot",
                                   name=f"ot{sqt}_{gb}")
                if gb % 2 == 0:
                    nc.vector.tensor_mul(
                        ot[:], ps[:],
                        zs[:, 8 + sqt:9 + sqt].to_broadcast([P, NBg]))
                else:
                    # same multiply on Scalar so the two gb blocks of a
                    # tile don't serialize on Vector
                    nc.scalar.activation(
                        ot[:], ps[:],
                        mybir.ActivationFunctionType.Copy,
                        scale=zs[:, 8 + sqt:9 + sqt])
                nc.sync.dma_start(
                    y_d[sqt * P:(sqt + 1) * P, gb * NBg:(gb + 1) * NBg],
                    ot[:])

            def out_half(h):
                for t in range(HT):
                    sqt = h * HT + t
                    last = h == NH - 1 and t == HT - 1
                    pss = [ps_pool.tile([P, NBg], dt.float32, tag="ps",
                                        name=f"ps_o{sqt}_{i}")
                           for i in range(D // NBg)]
                    if last:
                        # final tile: serial gb chains so gb0's scale+DMA
                        # overlaps gb1's matmuls, shortening the tail
                        for gb in range(D // NBg):
                            for ds in range(DS):
                                nc.tensor.matmul(
                                    pss[gb][:],
                                    px[:, ds, sqt * P:(sqt + 1) * P],
                                    w2T[:, ds, gb * NBg:(gb + 1) * NBg],
                                    start=(ds == 0), stop=(ds == DS - 1),
                                )
                            emit_out_scale(sqt, gb, pss[gb])
                    else:
                        for ds in range(DS):
                            lhs = px[:, ds, sqt * P:(sqt + 1) * P]
                            for gb in range(D // NBg):
                                nc.tensor.matmul(
                                    pss[gb][:], lhs,
                                    w2T[:, ds, gb * NBg:(gb + 1) * NBg],
                                    start=(ds == 0), stop=(ds == DS - 1),
                                )
                        for gb in range(D // NBg):
                            emit_out_scale(sqt, gb, pss[gb])

            ut_half(0)
            scores_half(0)
            ut_half(1)
            z_half(0)
            px_half(0)
            scores_half(1)
            out_half(0)
            z_half(1)
            px_half(1)
            out_half(1)

    nc.compile()
    _BUILD_CACHE[key] = nc
    return nc


def _run(x, wq, wk, wv, wo, trace=False):
    from concourse.bass_utils import run_bass_kernel_spmd

    B, S, D = x.shape
    SQ = B * S // N_CORES
    halves = S // SQ
    DS = D // P
    SKT = S // P
    nc = _build(S, D, SQ)

    bf = ml_dtypes.bfloat16
    x = np.asarray(x, dtype=np.float32)
    M = (np.asarray(wq, np.float32).T @ np.asarray(wk, np.float32))
    W2 = (np.asarray(wo, np.float32) @ np.asarray(wv, np.float32))
    # [k, n] -> [128, k/128, n] (k on partitions)
    mT = np.ascontiguousarray(
        M.reshape(DS, P, D).transpose(1, 0, 2)).astype(bf)
    w2T = np.ascontiguousarray(
        W2.T.reshape(DS, P, D).transpose(1, 0, 2)).astype(bf)

    in_maps = []
    for c in range(N_CORES):
        b, h = divmod(c, halves)
        xb = x[b]
        if h != 0:
            xb = np.concatenate([xb[h * SQ:(h + 1) * SQ], xb[:h * SQ],
                                 xb[(h + 1) * SQ:]], axis=0)
        xb = np.ascontiguousarray(xb, dtype=np.float32)
        # x.T over all keys, d on partitions: [128, DS, S]
        xT = np.ascontiguousarray(
            xb.T.reshape(DS, P, S).transpose(1, 0, 2)).astype(bf)
        # natural x, keys on partitions: [128, SKT, D]
        xn = np.ascontiguousarray(
            xb.reshape(SKT, P, D).transpose(1, 0, 2)).astype(bf)
        in_maps.append({"xT": xT, "xn": xn, "mT": mT, "w2T": w2T})

    res = run_bass_kernel_spmd(nc, in_maps, core_ids=list(range(N_CORES)),
                               trace=trace)
    out = np.empty((B, S, D), dtype=np.float32)
    for c in range(N_CORES):
        b, h = divmod(c, halves)
        out[b, h * SQ:(h + 1) * SQ, :] = res.results[c]["y"]
    return out, res


def kernel(x, wq, wk, wv, wo):
    out, _ = _run(x, wq, wk, wv, wo)
    return out

